# revision 4
# baseline (speedup 1.0000x reference)
"""Trainium2 Bass kernel for multi-head attention (B=4, N=2048, C=512, H=8, hd=64).

Returns (out, k, v) like the reference:
    qkv = x @ W_qkv -> q,k,v [B,H,N,hd]
    attn = softmax(q k^T / sqrt(hd)); out = (attn @ v) @ W_proj + b_proj

Sharding over 8 cores: core c handles batch b=c//2 and 4 heads hs=(c%2)*4.
Each core computes its batch's qkv projection (its head-columns only), the
attention for its 4 heads (flash-style, scores never leave the chip, computed
in transposed [key, query] layout so the softmax denominator falls out of the
attn@v matmul via an appended ones-row in v), and a partial output projection
(its 256 rows of W_proj). Host sums the two partial outputs per batch and adds
the bias.
"""

import numpy as np

B, N, C = 4, 2048, 512
H, HD = 8, 64
SCALE = HD ** -0.5
NB = N // 128      # 16 key/row blocks
CB = C // 128      # 4 contraction chunks
NQ = N // 512      # 4 moving-dim chunks
H_LOC = 4          # heads per core

_CACHED_NC = None


def _build_nc():
    from contextlib import ExitStack

    import concourse.tile as tile
    from concourse import bacc, mybir
    from concourse.masks import make_identity

    f32 = mybir.dt.float32

    nc = bacc.Bacc(
        "TRN2",
        target_bir_lowering=False,
        debug=False,
        enable_asserts=False,
        num_devices=8,
    )

    x_d = nc.dram_tensor("x", [N, C], f32, kind="ExternalInput").ap()
    wqk_d = nc.dram_tensor("wqk", [C, C], f32, kind="ExternalInput").ap()
    wv_d = nc.dram_tensor("wv", [C, H_LOC * HD], f32, kind="ExternalInput").ap()
    wp_d = nc.dram_tensor("wp", [H_LOC * HD, C], f32, kind="ExternalInput").ap()
    out_d = nc.dram_tensor("out_part", [N, C], f32, kind="ExternalOutput").ap()
    kT_d = nc.dram_tensor("kT_out", [H_LOC * HD, N], f32, kind="ExternalOutput").ap()
    v_d = nc.dram_tensor("v_out", [H_LOC, N, HD], f32, kind="ExternalOutput").ap()

    with tile.TileContext(nc) as tc, ExitStack() as ctx:
        singles = ctx.enter_context(tc.tile_pool(name="singles", bufs=1))

        ident = singles.tile([128, 128], f32)
        make_identity(nc, ident)
        ones_sb = singles.tile([128, 64], f32)
        nc.vector.memset(ones_sb[:], 1.0)

        wqk_sb = singles.tile([128, CB, C], f32)
        for cb in range(CB):
            nc.sync.dma_start(wqk_sb[:, cb, :], wqk_d[cb * 128:(cb + 1) * 128, :])
        wv_sb = singles.tile([128, CB, H_LOC * HD], f32)
        for cb in range(CB):
            nc.sync.dma_start(wv_sb[:, cb, :], wv_d[cb * 128:(cb + 1) * 128, :])
        wp_sb = singles.tile([128, 2, C], f32)
        for pb in range(2):
            nc.sync.dma_start(wp_sb[:, pb, :], wp_d[pb * 128:(pb + 1) * 128, :])

        xT_sb = singles.tile([128, CB, N], f32)       # x^T  [c, n]
        qkT_sb = singles.tile([128, 4, N], f32)       # blocks: q01 q23 k01 k23
        vaug_sb = singles.tile([128, NB, H_LOC, HD + 1], f32)  # v + ones col
        attnT_sb = singles.tile([128, 2, N], f32)     # attn-out^T per head pair

        # ones column (index HD) of vaug
        nc.gpsimd.memset(vaug_sb[:, :, :, HD:HD + 1], 1.0)

        # ---- Stage A: load x, transpose via PE ----
        with tc.tile_pool(name="xload", bufs=4) as xpool, \
             tc.tile_pool(name="ps_a", bufs=4, space="PSUM") as psa:
            for nb in range(NB):
                xt = xpool.tile([128, C], f32, tag="x")
                nc.sync.dma_start(xt[:], x_d[nb * 128:(nb + 1) * 128, :])
                for cb in range(CB):
                    ps = psa.tile([128, 128], f32, tag="tp")
                    nc.tensor.transpose(ps[:], xt[:, cb * 128:(cb + 1) * 128], ident[:])
                    nc.vector.tensor_copy(xT_sb[:, cb, nb * 128:(nb + 1) * 128], ps[:])

        # ---- Stage B: qT / kT projection (head pairs stacked on partitions) ----
        with tc.tile_pool(name="ps_b", bufs=3, space="PSUM") as psb:
            for blk in range(4):
                for nq in range(NQ):
                    ps = psb.tile([128, 512], f32, tag="proj")
                    for cb in range(CB):
                        nc.tensor.matmul(
                            ps[:],
                            wqk_sb[:, cb, blk * 128:(blk + 1) * 128],
                            xT_sb[:, cb, nq * 512:(nq + 1) * 512],
                            start=(cb == 0),
                            stop=(cb == CB - 1),
                        )
                    nc.vector.tensor_copy(qkT_sb[:, blk, nq * 512:(nq + 1) * 512], ps[:])
            for pb in range(2):
                nc.sync.dma_start(kT_d[pb * 128:(pb + 1) * 128, :], qkT_sb[:, 2 + pb, :])

            # ---- Stage C: v projection (natural layout, xT blocks as weights) ----
            for nb in range(NB):
                ps = psb.tile([128, H_LOC * HD], f32, tag="vproj")
                for cb in range(CB):
                    nc.tensor.matmul(
                        ps[:],
                        xT_sb[:, cb, nb * 128:(nb + 1) * 128],
                        wv_sb[:, cb, :],
                        start=(cb == 0),
                        stop=(cb == CB - 1),
                    )
                nc.vector.tensor_copy(
                    vaug_sb[:, nb, :, 0:HD],
                    ps[:].rearrange("p (h d) -> p h d", h=H_LOC),
                )
                nc.sync.dma_start(
                    v_d[:, nb * 128:(nb + 1) * 128, :].rearrange("h p d -> p h d"),
                    vaug_sb[:, nb, :, 0:HD],
                )

        # ---- Stage D: attention, transposed scores, flash-style ----
        with tc.tile_pool(name="ps_s", bufs=4, space="PSUM") as pss, \
             tc.tile_pool(name="ps_o", bufs=3, space="PSUM") as pso, \
             tc.tile_pool(name="ps_bc", bufs=1, space="PSUM") as psbc, \
             tc.tile_pool(name="expp", bufs=6) as expp, \
             tc.tile_pool(name="dnp", bufs=4) as dnp:
            for pb in range(2):
                qT2 = qkT_sb[:, pb, :]
                kT2 = qkT_sb[:, 2 + pb, :]
                for iq in range(NQ):
                    isl = slice(iq * 512, (iq + 1) * 512)
                    out_ps = [
                        pso.tile([128, 512], f32, tag="avout", name=f"avout_{pb}_{iq}_0"),
                        pso.tile([128, 512], f32, tag="avout", name=f"avout_{pb}_{iq}_1"),
                    ]
                    for jb in range(NB):
                        jsl = slice(jb * 128, (jb + 1) * 128)
                        e_sb = [None, None]
                        for a in range(2):
                            asl = slice(a * 64, (a + 1) * 64)
                            s_ps = pss.tile([128, 512], f32, tag="s")
                            # scores^T[j, i] for head a of the pair (K=64,
                            # row-tiled: partitions a*64..a*64+63)
                            nc.tensor.matmul(
                                s_ps[:], kT2[asl, jsl], qT2[asl, isl],
                                start=True, stop=True,
                            )
                            e = expp.tile([128, 512], f32, tag="e")
                            nc.scalar.activation(
                                e[:], s_ps[:],
                                func=mybir.ActivationFunctionType.Exp,
                                scale=SCALE,
                            )
                            e_sb[a] = e
                        for a in range(2):
                            # attn-out^T accumulate; row 64 = sum(exp) (ones col)
                            nc.tensor.matmul(
                                out_ps[a][0:65, :],
                                vaug_sb[:, jb, 2 * pb + a, :],
                                e_sb[a][:],
                                start=(jb == 0),
                                stop=(jb == NB - 1),
                                skip_group_check=True,
                            )
                    for a in range(2):
                        # normalize: bcast denom via K=1 matmul, recip, mult
                        den = dnp.tile([128, 512], f32, tag="den")
                        nc.vector.tensor_copy(den[64:65, :], out_ps[a][64:65, :])
                        bc_ps = psbc.tile([128, 512], f32, tag="bc")
                        nc.tensor.matmul(
                            bc_ps[0:64, :], ones_sb[64:65, :], den[64:65, :],
                            start=True, stop=True,
                        )
                        rec = dnp.tile([64, 512], f32, tag="rec")
                        nc.vector.reciprocal(rec[:], bc_ps[0:64, :])
                        nc.vector.tensor_mul(
                            attnT_sb[a * 64:(a + 1) * 64, pb, isl],
                            out_ps[a][0:64, :],
                            rec[:],
                        )

        # ---- Stage E: output projection ----
        with tc.tile_pool(name="ps_e", bufs=4, space="PSUM") as pse, \
             tc.tile_pool(name="outp", bufs=4) as outp:
            for ib in range(NB):
                ps = pse.tile([128, 512], f32, tag="proj_o")
                for pb in range(2):
                    nc.tensor.matmul(
                        ps[:],
                        attnT_sb[:, pb, ib * 128:(ib + 1) * 128],
                        wp_sb[:, pb, :],
                        start=(pb == 0),
                        stop=(pb == 1),
                    )
                o_sb = outp.tile([128, 512], f32, tag="o")
                nc.vector.tensor_copy(o_sb[:], ps[:])
                nc.sync.dma_start(out_d[ib * 128:(ib + 1) * 128, :], o_sb[:])

    nc.compile()
    return nc


def get_nc():
    global _CACHED_NC
    if _CACHED_NC is None:
        _CACHED_NC = _build_nc()
    return _CACHED_NC


def make_in_maps(x, W_qkv, W_proj):
    x = np.asarray(x, dtype=np.float32)
    W_qkv = np.asarray(W_qkv, dtype=np.float32)
    W_proj = np.asarray(W_proj, dtype=np.float32)
    in_maps = []
    for core in range(8):
        b = core // 2
        hs = (core % 2) * H_LOC
        heads = range(hs, hs + H_LOC)
        qcols = np.concatenate([W_qkv[:, h * HD:(h + 1) * HD] for h in heads], axis=1)
        kcols = np.concatenate(
            [W_qkv[:, C + h * HD:C + (h + 1) * HD] for h in heads], axis=1)
        vcols = np.concatenate(
            [W_qkv[:, 2 * C + h * HD:2 * C + (h + 1) * HD] for h in heads], axis=1)
        in_maps.append({
            "x": np.ascontiguousarray(x[b]),
            "wqk": np.ascontiguousarray(np.concatenate([qcols, kcols], axis=1)),
            "wv": np.ascontiguousarray(vcols),
            "wp": np.ascontiguousarray(W_proj[hs * HD:(hs + H_LOC) * HD, :]),
        })
    return in_maps


def assemble(results, b_proj):
    b_proj = np.asarray(b_proj, dtype=np.float32)
    out = np.empty((B, N, C), dtype=np.float32)
    k = np.empty((B, H, N, HD), dtype=np.float32)
    v = np.empty((B, H, N, HD), dtype=np.float32)
    for b in range(B):
        out[b] = (results[2 * b]["out_part"] + results[2 * b + 1]["out_part"]
                  + b_proj[None, :])
    for core in range(8):
        b = core // 2
        hs = (core % 2) * H_LOC
        kT = results[core]["kT_out"]  # [256, N]
        k[b, hs:hs + H_LOC] = kT.reshape(H_LOC, HD, N).transpose(0, 2, 1)
        v[b, hs:hs + H_LOC] = results[core]["v_out"]
    return out, k, v


def run(x, W_qkv, W_proj, b_proj, trace=False):
    from concourse.bass_utils import run_bass_kernel_spmd

    nc = get_nc()
    in_maps = make_in_maps(x, W_qkv, W_proj)
    res = run_bass_kernel_spmd(nc, in_maps, core_ids=list(range(8)), trace=trace)
    out, k, v = assemble(res.results, b_proj)
    return (out, k, v), res


def kernel(x, W_qkv, W_proj, b_proj):
    (out, k, v), _ = run(x, W_qkv, W_proj, b_proj, trace=False)
    return out, k, v
